# revision 1
# baseline (speedup 1.0000x reference)
"""LogEig Trainium2 kernel v4: block-diagonal pair packing, tuned dataflow.

vs v3: arena block copies run on ACT (top) + DVE (bottom) with low-latency
semaphores instead of DMA; fit accumulation runs on PE as wide fp32r
matmuls with constant scaled-identity stationaries (CI) into an
accumulating PSUM bank; F2 folded via pre-scaled D_8 arena.
"""

import numpy as np

import concourse.bass as bass
import concourse.mybir as mybir
from concourse import bacc
from concourse.bass import ds
from concourse.bass_utils import run_bass_kernel_spmd
from concourse.tile import TileContext

F32 = mybir.dt.float32
F32R = mybir.dt.float32r
ALU = mybir.AluOpType

SIGMAS = [-1.075177135e-01, -2.867541926e-01, -7.649643581e-01,
          -2.041943548e+00, -5.459703523e+00, -1.466292403e+01,
          -3.984730093e+01, -1.117281157e+02]
C0 = -8.102624854e+00
C_D = [2.196022600e-01, 4.297179445e-01, 1.210738248e+00, 3.196899612e+00,
       8.584713458e+00, 2.307315480e+01, 6.323317755e+01, 1.819681532e+02,
       1.337874966e+03]
CF2EFF = -5.832597604e+02 * 4.469124630e+02

K_STEPS = 8
N_MAT = 1024
BLK = 16
NPAIR = BLK // 2
INTERLEAVE = 4
USE_F32R = True


def _arena_views(AR):
    top = AR[0:64, :].rearrange("p (b c) -> p b c", c=128)[:, :, 0:64]
    bot = AR[64:128, :].rearrange("p (b c) -> p b c", c=128)[:, :, 64:128]
    return top, bot


def _deck_views(D):
    t = D[0:64, :].rearrange("p (b c) -> p b c", c=64)
    b = D[64:128, :].rearrange("p (b c) -> p b c", c=64)
    return t, b


def _mm_sq_blk(nc, psum, AR, D, start=True, stop=True):
    for p in range(NPAIR):
        nc.tensor.matmul(psum[0:128, ds(64 * p, 64)],
                         AR[0:128, ds(128 * p, 128)],
                         D[0:128, ds(64 * p, 64)],
                         start=start, stop=stop, skip_group_check=True)


def _acc_term(nc, ACC, CI, C2, j, D, start, stop):
    if j >= K_STEPS - 2:
        lhsT = C2[:, ds(128 * (j - (K_STEPS - 2)), 128)]
    else:
        lhsT = CI[:, ds(128 * j, 128)]
    nc.tensor.matmul(ACC[:, :], lhsT, D[:, :],
                     start=start, stop=stop, skip_group_check=True)


def _emit_head(nc, pool, pspool, arenas, base, P_d, IC, par):
    ARin = arenas[par][2]            # F32 arena, level 0 only
    Din = pool.tile([128, 512], F32, tag=f"Din_{par}", name=f"Din_{par}")
    nc.sync.dma_start(Din[0:64, :], P_d[ds(base, 8)].transpose([1, 0, 2]))
    nc.sync.dma_start(Din[64:128, :], P_d[ds(base + 8, 8)].transpose([1, 0, 2]))
    art, arb = _arena_views(ARin)
    nc.sync.dma_start(art, P_d[ds(base, 8)].transpose([1, 0, 2]))
    nc.sync.dma_start(arb, P_d[ds(base + 8, 8)].transpose([1, 0, 2]))
    T = pool.tile([128, 512], F32, tag=f"T_{par}", name=f"T_{par}")
    nc.vector.scalar_tensor_tensor(T, Din, float(C_D[0]), IC,
                                   ALU.mult, ALU.add)
    ACC = pspool.tile([128, 512], F32, tag=f"acc{par}")
    return dict(D=Din, AR=ARin, T=T, ACC=ACC)


def _emit_step(nc, pool, pspool, arenas, st, CI, C2, par, i):
    D, AR, ACC = st["D"], st["AR"], st["ACC"]
    ps = pspool.tile([128, 512], F32, tag=f"ps{par}", name=f"ps{par}")
    _mm_sq_blk(nc, ps, AR, D)
    if i >= K_STEPS - 3:
        Dn = pool.tile([128, 512], mybir.dt.bfloat16, tag=f"Db{(i + 1) % 2}_{par}",
                       name=f"Db{(i + 1) % 2}_{par}")
    else:
        Dn = pool.tile([128, 512], F32R, tag=f"D{(i + 1) % 2}_{par}",
                       name=f"D{(i + 1) % 2}_{par}")
    nc.vector.scalar_tensor_tensor(Dn, ps, float(SIGMAS[i]), D,
                                   ALU.mult, ALU.add)
    D = Dn
    if i >= K_STEPS - 3:
        ARn = arenas[par][3 + (i + 1) % 2]
    else:
        ARn = arenas[par][(i + 1) % 2]
    dt, db = _deck_views(D)
    art, arb = _arena_views(ARn)
    scale = CF2EFF if i == K_STEPS - 1 else 1.0
    nc.scalar.mul(art, dt, float(scale))
    nc.vector.tensor_scalar_mul(arb, db, float(scale))
    _acc_term(nc, ACC, CI, C2, i + 1, D, start=(i == 0), stop=False)
    st["D"], st["AR"] = D, ARn


def _emit_tail(nc, pool, st, base, O_d, par):
    D, AR, ACC, T = st["D"], st["AR"], st["ACC"], st["T"]
    _mm_sq_blk(nc, ACC, AR, D, start=False, stop=True)
    OUT = pool.tile([128, 512], F32, tag=f"OUT_{par}", name=f"OUT_{par}")
    nc.vector.scalar_tensor_tensor(OUT, ACC, 1.0, T, ALU.mult, ALU.add)
    nc.sync.dma_start(O_d[ds(base, 8)].transpose([1, 0, 2]), OUT[0:64, :])
    nc.sync.dma_start(O_d[ds(base + 8, 8)].transpose([1, 0, 2]), OUT[64:128, :])


def build_nc(n_mat=N_MAT, unroll=False):
    nc = bacc.Bacc("TRN2", target_bir_lowering=False, debug=False,
                   num_devices=8)
    P_d = nc.dram_tensor("P", [n_mat, 64, 64], F32, kind="ExternalInput").ap()
    O_d = nc.dram_tensor("OUT", [n_mat, 64, 64], F32, kind="ExternalOutput").ap()
    IC_d = nc.dram_tensor("IC", [128, 512], F32, kind="ExternalInput").ap()
    ZR_d = nc.dram_tensor("ZR", [128, 128 * NPAIR], F32R,
                          kind="ExternalInput").ap()
    C2_d = nc.dram_tensor("C2", [128, 384], mybir.dt.bfloat16,
                          kind="ExternalInput").ap()
    CI_d = nc.dram_tensor("CI", [128, 128 * (K_STEPS + 1)], F32R,
                          kind="ExternalInput").ap()
    with TileContext(nc) as tc:
        with (
            tc.tile_pool(name="consts", bufs=1) as cpool,
            tc.tile_pool(name="work", bufs=2) as pool,
            tc.tile_pool(name="psum", bufs=1, space=bass.MemorySpace.PSUM) as pspool,
            tc.tile_pool(name="psacc", bufs=1, space=bass.MemorySpace.PSUM) as paccpool,
        ):
            IC = cpool.tile([128, 512], F32)
            nc.sync.dma_start(IC[:], IC_d)
            CI = cpool.tile([128, 128 * (K_STEPS + 1)], F32R)
            nc.sync.dma_start(CI[:], CI_d)
            C2 = cpool.tile([128, 384], mybir.dt.bfloat16)
            nc.sync.dma_start(C2[:], C2_d)
            arenas = []
            for par in range(INTERLEAVE):
                group = []
                for k in range(2):
                    AR = cpool.tile([128, 128 * NPAIR], F32R, tag=f"AR{k}_{par}")
                    nc.sync.dma_start(AR[:], ZR_d)
                    group.append(AR)
                ARi = cpool.tile([128, 128 * NPAIR], F32, tag=f"ARin_{par}")
                nc.gpsimd.memset(ARi[:], 0.0)
                group.append(ARi)
                for k in range(2):
                    ARb = cpool.tile([128, 128 * NPAIR], mybir.dt.bfloat16,
                                     tag=f"ARb{k}_{par}")
                    nc.gpsimd.memset(ARb[:], 0.0)
                    group.append(ARb)
                arenas.append(group)

            def ps_pool_shim(shape, dtype, tag):
                if tag.startswith("acc"):
                    return paccpool.tile(shape, dtype, tag=tag, name=tag)
                return pspool.tile(shape, dtype, tag=tag, name=tag)

            class _PS:
                def tile(self, shape, dtype, tag, name=None):
                    return ps_pool_shim(shape, dtype, tag)

            psp = _PS()
            def body(m0):
                sts = []
                for par in range(INTERLEAVE):
                    sts.append(_emit_head(nc, pool, psp, arenas,
                                          m0 + par * BLK, P_d, IC, par))
                for i in range(K_STEPS):
                    for par in range(INTERLEAVE):
                        _emit_step(nc, pool, psp, arenas, sts[par], CI, C2, par, i)
                for par in range(INTERLEAVE):
                    _emit_tail(nc, pool, sts[par], m0 + par * BLK, O_d, par)

            step = BLK * INTERLEAVE
            if unroll:
                for m0 in range(0, n_mat, step):
                    body(m0)
            else:
                with tc.For_i(0, n_mat, step) as m0:
                    body(m0)
    nc.compile()
    return nc


def _ic_const():
    ic = np.zeros((128, 512), np.float32)
    for p in range(128):
        for k in range(8):
            ic[p, 64 * k + (p % 64)] = C0
    return ic


def _ci_const():
    ci = np.zeros((128, 128 * (K_STEPS + 1)), np.float32)
    for j, c in enumerate(C_D):
        for p in range(128):
            ci[p, 128 * j + p] = c
    return ci


def _c2_const():
    import ml_dtypes
    c2 = np.zeros((128, 384), ml_dtypes.bfloat16)
    for k, c in enumerate(C_D[K_STEPS - 2:K_STEPS + 1]):
        for p in range(128):
            c2[p, 128 * k + p] = c
    return c2


def host_constants():
    return {"IC": _ic_const(), "CI": _ci_const(), "C2": _c2_const(),
            "ZR": np.zeros((128, 128 * NPAIR), np.float32)}


_NC_CACHE = {}


def kernel(P: np.ndarray) -> np.ndarray:
    P = np.ascontiguousarray(np.asarray(P), dtype=np.float32)
    B, H, N, _ = P.shape
    flat = P.reshape(-1, N, N)
    n_cores = 8
    per = flat.shape[0] // n_cores
    if "nc" not in _NC_CACHE:
        _NC_CACHE["nc"] = build_nc()
    nc = _NC_CACHE["nc"]
    consts = host_constants()
    in_maps = [
        {"P": np.ascontiguousarray(flat[c * per:(c + 1) * per]), **consts}
        for c in range(n_cores)
    ]
    res = run_bass_kernel_spmd(nc, in_maps, core_ids=list(range(n_cores)))
    out = np.concatenate([r["OUT"] for r in res.results], axis=0)
    return out.reshape(B, H, N, N).astype(np.float32)



# revision 4
# speedup vs baseline: 102.5766x; 102.5766x over previous
"""LogEig Trainium2 kernel v7: fp16 state + operands.

fp16 (11-bit mantissa ~ tf32 precision) is matmul-ready at full PE
rate, so the iterate state doubles as both squaring operands -- no
per-round cast ops at all. Block-diag fp16 arenas; arena copies on
ACT; state STT + output copy on DVE; acc matmuls (fp16 diag
stationaries) and C0*I on PE. gpsimd only for preamble memsets
(HW gpsimd elementwise is ~6us/op -- never in the loop).
"""

import numpy as np

import concourse.bass as bass
import concourse.mybir as mybir
from concourse import bacc
from concourse.bass import ds
from concourse.bass_utils import run_bass_kernel_spmd
from concourse.tile import TileContext

F32 = mybir.dt.float32
FP16 = mybir.dt.float16
ALU = mybir.AluOpType

SIGMAS = [-1.075177135e-01, -2.867541926e-01, -7.649643581e-01,
          -2.041943548e+00, -5.459703523e+00, -1.466292403e+01,
          -3.984730093e+01, -1.117281157e+02]
C0 = -8.102624854e+00
C_D = [2.196022600e-01, 4.297179445e-01, 1.210738248e+00, 3.196899612e+00,
       8.584713458e+00, 2.307315480e+01, 6.323317755e+01, 1.819681532e+02,
       1.337874966e+03]
CF2EFF = -5.832597604e+02 * 4.469124630e+02

K_STEPS = 8
N_MAT = 1024
BLK = 16
NPAIR = BLK // 2
INTERLEAVE = 4


def _arena_views(AR):
    top = AR[0:64, :].rearrange("p (b c) -> p b c", c=128)[:, :, 0:64]
    bot = AR[64:128, :].rearrange("p (b c) -> p b c", c=128)[:, :, 64:128]
    return top, bot


def _mm_sq_blk(nc, psum, AR, RHS, start=True, stop=True):
    for p in range(NPAIR):
        nc.tensor.matmul(psum[0:128, ds(64 * p, 64)],
                         AR[0:128, ds(128 * p, 128)],
                         RHS[0:128, ds(64 * p, 64)],
                         start=start, stop=stop, skip_group_check=True)


def build_nc(n_mat=N_MAT, unroll=False, nrep=1):
    nc = bacc.Bacc("TRN2", target_bir_lowering=False, debug=False,
                   num_devices=8)
    PH_d = nc.dram_tensor("PH", [n_mat, 64, 64], FP16,
                          kind="ExternalInput").ap()
    O_d = nc.dram_tensor("OUT", [n_mat, 64, 64], F32,
                         kind="ExternalOutput").ap()
    # CIH: fp16 diag stationaries: c_0..c_8 then C0 (10 blocks of 128).
    # IDT: fp16 block-diag identity pattern (moving operand for C0*I).
    CIH_d = nc.dram_tensor("CIH", [128, 128 * (K_STEPS + 2)], FP16,
                           kind="ExternalInput").ap()
    IDT_d = nc.dram_tensor("IDT", [128, 512], FP16,
                           kind="ExternalInput").ap()
    with TileContext(nc) as tc:
        with (
            tc.tile_pool(name="consts", bufs=1) as cpool,
            tc.tile_pool(name="work", bufs=2) as pool,
            tc.tile_pool(name="psum", bufs=1, space=bass.MemorySpace.PSUM) as pspool,
            tc.tile_pool(name="psacc", bufs=1, space=bass.MemorySpace.PSUM) as paccpool,
        ):
            CIH = cpool.tile([128, 128 * (K_STEPS + 2)], FP16)
            nc.sync.dma_start(CIH[:], CIH_d)
            IDT = cpool.tile([128, 512], FP16)
            nc.sync.dma_start(IDT[:], IDT_d)

            arenas = []
            for par in range(INTERLEAVE):
                group = []
                for k in range(2):
                    AR = cpool.tile([128, 128 * NPAIR], FP16,
                                    tag=f"AR{k}_{par}", name=f"AR{k}_{par}")
                    nc.gpsimd.memset(AR[:], 0.0)
                    group.append(AR)
                arenas.append(group)

            def emit_group(par, base):
                # -- head --
                D = pool.tile([128, 512], FP16, tag=f"D0_{par}",
                              name=f"D0_{par}")
                src = PH_d[ds(base, BLK)]
                nc.sync.dma_start(
                    D[:], src.rearrange("(b h) r c -> (h r) b c", h=2))
                ar0 = arenas[par][0]
                art, arb = _arena_views(ar0)
                ev = src.rearrange("(b h) r c -> h r b c", h=2)
                nc.sync.dma_start(art, ev[0])
                nc.sync.dma_start(arb, ev[1])
                ACC = paccpool.tile([128, 512], F32, tag=f"acc{par}",
                                    name=f"acc{par}")
                nc.tensor.matmul(ACC[:, :],
                                 CIH[:, ds(128 * (K_STEPS + 1), 128)],
                                 IDT[:, :], start=True, stop=False,
                                 skip_group_check=True)
                nc.tensor.matmul(ACC[:, :], CIH[:, ds(0, 128)], D[:, :],
                                 start=False, stop=False,
                                 skip_group_check=True)
                # -- rounds --
                for j in range(K_STEPS):
                    ps = pspool.tile([128, 512], F32, tag=f"ps{par}",
                                     name=f"ps{par}")
                    _mm_sq_blk(nc, ps, arenas[par][j % 2], D)
                    nxt = j + 1
                    Dn = pool.tile([128, 512], FP16, tag=f"D{nxt % 2}_{par}",
                                   name=f"D{nxt % 2}_{par}")
                    nc.vector.scalar_tensor_tensor(Dn, ps, float(SIGMAS[j]),
                                                   D, ALU.mult, ALU.add)
                    ARn = arenas[par][nxt % 2]
                    art, arb = _arena_views(ARn)
                    scale = CF2EFF if j == K_STEPS - 1 else 1.0
                    if j % 2 == 0:
                        nc.scalar.mul(art, Dn[0:64, :], float(scale))
                        nc.vector.tensor_scalar_mul(arb, Dn[64:128, :],
                                                    float(scale))
                    else:
                        nc.vector.tensor_scalar_mul(art, Dn[0:64, :],
                                                    float(scale))
                        nc.scalar.mul(arb, Dn[64:128, :], float(scale))
                    D = Dn
                    nc.tensor.matmul(ACC[:, :], CIH[:, ds(128 * nxt, 128)],
                                     D[:, :], start=False, stop=False,
                                     skip_group_check=True)
                # -- tail --
                _mm_sq_blk(nc, ACC, arenas[par][K_STEPS % 2], D,
                           start=False, stop=True)
                OT = pool.tile([128, 512], F32, tag=f"OT_{par}",
                               name=f"OT_{par}")
                if par % 2 == 0:
                    nc.vector.tensor_copy(OT, ACC)
                else:
                    nc.scalar.copy(OT, ACC)
                dst = O_d[ds(base, BLK)]
                nc.scalar.dma_start(
                    dst.rearrange("(b h) r c -> (h r) b c", h=2), OT[:])

            def body(m0):
                for par in range(INTERLEAVE):
                    emit_group(par, m0 + par * BLK)

            step = BLK * INTERLEAVE
            if unroll:
                for m0 in range(0, n_mat, step):
                    body(m0)
            elif nrep == 1:
                with tc.For_i(0, n_mat, step) as m0:
                    body(m0)
            else:
                with tc.For_i(0, nrep) as _r:
                    with tc.For_i(0, n_mat, step) as m0:
                        body(m0)
    nc.compile()
    return nc


def _cih_const():
    ci = np.zeros((128, 128 * (K_STEPS + 2)), np.float16)
    for j in range(K_STEPS + 1):
        for p in range(128):
            ci[p, 128 * j + p] = np.float16(C_D[j])
    for p in range(128):
        ci[p, 128 * (K_STEPS + 1) + p] = np.float16(C0)
    return ci


def _idt_const():
    idt = np.zeros((128, 512), np.float16)
    for p in range(128):
        for k in range(8):
            idt[p, 64 * k + (p % 64)] = 1.0
    return idt


def host_constants():
    return {"CIH": _cih_const(), "IDT": _idt_const()}


_NC_CACHE = {}


def kernel(P: np.ndarray) -> np.ndarray:
    P = np.asarray(P)
    B, H, N, _ = P.shape
    flath = np.ascontiguousarray(
        P.reshape(-1, N, N).astype(np.float16))
    n_cores = 8
    per = flath.shape[0] // n_cores
    if "nc" not in _NC_CACHE:
        _NC_CACHE["nc"] = build_nc()
    nc = _NC_CACHE["nc"]
    consts = host_constants()
    in_maps = [
        {"PH": flath[c * per:(c + 1) * per], **consts}
        for c in range(n_cores)
    ]
    res = run_bass_kernel_spmd(nc, in_maps, core_ids=list(range(n_cores)))
    out = np.concatenate([r["OUT"] for r in res.results], axis=0)
    return out.reshape(B, H, N, N).astype(np.float32)


# revision 5
# speedup vs baseline: 110.4729x; 1.0770x over previous
"""LogEig Trainium2 kernel v8: fp16 state, hybrid blockdiag/quad squarings.

fp16 (11-bit mantissa ~ tf32 precision) is matmul-ready at full PE
rate, so the iterate state doubles as both squaring operands -- no
per-round cast ops at all. Block-diag fp16 arenas; arena copies on
ACT; state STT + output copy on DVE; acc matmuls (fp16 diag
stationaries) and C0*I on PE. gpsimd only for preamble memsets
(HW gpsimd elementwise is ~6us/op -- never in the loop).
"""

import numpy as np

import concourse.bass as bass
import concourse.mybir as mybir
from concourse import bacc
from concourse.bass import ds
from concourse.bass_utils import run_bass_kernel_spmd
from concourse.tile import TileContext

F32 = mybir.dt.float32
FP16 = mybir.dt.float16
ALU = mybir.AluOpType

SIGMAS = [-1.075177135e-01, -2.867541926e-01, -7.649643581e-01,
          -2.041943548e+00, -5.459703523e+00, -1.466292403e+01,
          -3.984730093e+01, -1.117281157e+02]
C0 = -8.102624854e+00
C_D = [2.196022600e-01, 4.297179445e-01, 1.210738248e+00, 3.196899612e+00,
       8.584713458e+00, 2.307315480e+01, 6.323317755e+01, 1.819681532e+02,
       1.337874966e+03]
CF2EFF = -5.832597604e+02 * 4.469124630e+02

K_STEPS = 8
N_MAT = 1024
BLK = 16
NPAIR = BLK // 2
INTERLEAVE = 4
QF = 4     # rounds >= QF use quad-tiled matmuls on the deck (no arena)


def _arena_views(AR):
    top = AR[0:64, :].rearrange("p (b c) -> p b c", c=128)[:, :, 0:64]
    bot = AR[64:128, :].rearrange("p (b c) -> p b c", c=128)[:, :, 64:128]
    return top, bot


def _mm_sq_blk(nc, psum, AR, RHS, start=True, stop=True):
    for p in range(NPAIR):
        nc.tensor.matmul(psum[0:128, ds(64 * p, 64)],
                         AR[0:128, ds(128 * p, 128)],
                         RHS[0:128, ds(64 * p, 64)],
                         start=start, stop=stop, skip_group_check=True)


def _mm_sq_quad(nc, psum, LHS, RHS, start=True, stop=True):
    """Per-pair quadrant matmuls reading deck layout directly:
    top matrix in partitions 0:64, bottom in 64:128."""
    for p in range(NPAIR):
        nc.tensor.matmul(psum[0:64, ds(64 * p, 64)],
                         LHS[0:64, ds(64 * p, 64)],
                         RHS[0:64, ds(64 * p, 64)],
                         start=start, stop=stop, tile_position=(0, 0),
                         skip_group_check=True)
        nc.tensor.matmul(psum[64:128, ds(64 * p, 64)],
                         LHS[64:128, ds(64 * p, 64)],
                         RHS[64:128, ds(64 * p, 64)],
                         start=start, stop=stop, tile_position=(64, 64),
                         skip_group_check=True)


def build_nc(n_mat=N_MAT, unroll=False, nrep=1):
    nc = bacc.Bacc("TRN2", target_bir_lowering=False, debug=False,
                   num_devices=8)
    PH_d = nc.dram_tensor("PH", [n_mat, 64, 64], FP16,
                          kind="ExternalInput").ap()
    O_d = nc.dram_tensor("OUT", [n_mat, 64, 64], F32,
                         kind="ExternalOutput").ap()
    # CIH: fp16 diag stationaries: c_0..c_8 then C0 (10 blocks of 128).
    # IDT: fp16 block-diag identity pattern (moving operand for C0*I).
    CIH_d = nc.dram_tensor("CIH", [128, 128 * (K_STEPS + 2)], FP16,
                           kind="ExternalInput").ap()
    IDT_d = nc.dram_tensor("IDT", [128, 512], FP16,
                           kind="ExternalInput").ap()
    with TileContext(nc) as tc:
        with (
            tc.tile_pool(name="consts", bufs=1) as cpool,
            tc.tile_pool(name="work", bufs=2) as pool,
            tc.tile_pool(name="psum", bufs=1, space=bass.MemorySpace.PSUM) as pspool,
            tc.tile_pool(name="psacc", bufs=1, space=bass.MemorySpace.PSUM) as paccpool,
        ):
            CIH = cpool.tile([128, 128 * (K_STEPS + 2)], FP16)
            nc.sync.dma_start(CIH[:], CIH_d)
            IDT = cpool.tile([128, 512], FP16)
            nc.sync.dma_start(IDT[:], IDT_d)

            arenas = []
            for par in range(INTERLEAVE):
                group = []
                for k in range(2):
                    AR = cpool.tile([128, 128 * NPAIR], FP16,
                                    tag=f"AR{k}_{par}", name=f"AR{k}_{par}")
                    nc.gpsimd.memset(AR[:], 0.0)
                    group.append(AR)
                arenas.append(group)

            def emit_group(par, base):
                # -- head --
                D = pool.tile([128, 512], FP16, tag=f"D0_{par}",
                              name=f"D0_{par}")
                src = PH_d[ds(base, BLK)]
                nc.sync.dma_start(
                    D[:], src.rearrange("(b h) r c -> (h r) b c", h=2))
                ar0 = arenas[par][0]
                art, arb = _arena_views(ar0)
                ev = src.rearrange("(b h) r c -> h r b c", h=2)
                nc.sync.dma_start(art, ev[0])
                nc.sync.dma_start(arb, ev[1])
                ACC = paccpool.tile([128, 512], F32, tag=f"acc{par}",
                                    name=f"acc{par}")
                nc.tensor.matmul(ACC[:, :],
                                 CIH[:, ds(128 * (K_STEPS + 1), 128)],
                                 IDT[:, :], start=True, stop=False,
                                 skip_group_check=True)
                nc.tensor.matmul(ACC[:, :], CIH[:, ds(0, 128)], D[:, :],
                                 start=False, stop=False,
                                 skip_group_check=True)
                # -- rounds --
                for j in range(K_STEPS):
                    ps = pspool.tile([128, 512], F32, tag=f"ps{par}",
                                     name=f"ps{par}")
                    if j < QF:
                        _mm_sq_blk(nc, ps, arenas[par][j % 2], D)
                    else:
                        _mm_sq_quad(nc, ps, D, D)
                    nxt = j + 1
                    Dn = pool.tile([128, 512], FP16, tag=f"D{nxt % 2}_{par}",
                                   name=f"D{nxt % 2}_{par}")
                    nc.vector.scalar_tensor_tensor(Dn, ps, float(SIGMAS[j]),
                                                   D, ALU.mult, ALU.add)
                    D = Dn
                    if nxt < QF:
                        ARn = arenas[par][nxt % 2]
                        art, arb = _arena_views(ARn)
                        nc.scalar.mul(art, Dn[0:64, :], 1.0)
                        nc.scalar.mul(arb, Dn[64:128, :], 1.0)
                    nc.tensor.matmul(ACC[:, :], CIH[:, ds(128 * nxt, 128)],
                                     D[:, :], start=False, stop=False,
                                     skip_group_check=True)
                # -- tail: quad with cf2-scaled stationary copy --
                SCT = pool.tile([128, 512], FP16, tag=f"SC_{par}",
                                name=f"SC_{par}")
                nc.scalar.mul(SCT[:], D[:], float(CF2EFF))
                _mm_sq_quad(nc, ACC, SCT, D, start=False, stop=True)
                OT = pool.tile([128, 512], F32, tag=f"OT_{par}",
                               name=f"OT_{par}")
                nc.scalar.copy(OT, ACC)
                dst = O_d[ds(base, BLK)]
                nc.scalar.dma_start(
                    dst.rearrange("(b h) r c -> (h r) b c", h=2), OT[:])

            def body(m0):
                for par in range(INTERLEAVE):
                    emit_group(par, m0 + par * BLK)

            step = BLK * INTERLEAVE
            if unroll:
                for m0 in range(0, n_mat, step):
                    body(m0)
            elif nrep == 1:
                with tc.For_i(0, n_mat, step) as m0:
                    body(m0)
            else:
                with tc.For_i(0, nrep) as _r:
                    with tc.For_i(0, n_mat, step) as m0:
                        body(m0)
    nc.compile()
    return nc


def _cih_const():
    ci = np.zeros((128, 128 * (K_STEPS + 2)), np.float16)
    for j in range(K_STEPS + 1):
        for p in range(128):
            ci[p, 128 * j + p] = np.float16(C_D[j])
    for p in range(128):
        ci[p, 128 * (K_STEPS + 1) + p] = np.float16(C0)
    return ci


def _idt_const():
    idt = np.zeros((128, 512), np.float16)
    for p in range(128):
        for k in range(8):
            idt[p, 64 * k + (p % 64)] = 1.0
    return idt


def host_constants():
    return {"CIH": _cih_const(), "IDT": _idt_const()}


_NC_CACHE = {}


def kernel(P: np.ndarray) -> np.ndarray:
    P = np.asarray(P)
    B, H, N, _ = P.shape
    flath = np.ascontiguousarray(
        P.reshape(-1, N, N).astype(np.float16))
    n_cores = 8
    per = flath.shape[0] // n_cores
    if "nc" not in _NC_CACHE:
        _NC_CACHE["nc"] = build_nc()
    nc = _NC_CACHE["nc"]
    consts = host_constants()
    in_maps = [
        {"PH": flath[c * per:(c + 1) * per], **consts}
        for c in range(n_cores)
    ]
    res = run_bass_kernel_spmd(nc, in_maps, core_ids=list(range(n_cores)))
    out = np.concatenate([r["OUT"] for r in res.results], axis=0)
    return out.reshape(B, H, N, N).astype(np.float32)


# revision 6
# speedup vs baseline: 113.5838x; 1.0282x over previous
"""LogEig Trainium2 kernel v10: fp16 state, hybrid blockdiag/quad squarings,
no final-square term (coefficients refit for the 8-product family).

fp16 (11-bit mantissa ~ tf32 precision) is matmul-ready at full PE
rate, so the iterate state doubles as both squaring operands -- no
per-round cast ops at all. Block-diag fp16 arenas; arena copies on
ACT; state STT + output copy on DVE; acc matmuls (fp16 diag
stationaries) and C0*I on PE. gpsimd only for preamble memsets
(HW gpsimd elementwise is ~6us/op -- never in the loop).
"""

import numpy as np

import concourse.bass as bass
import concourse.mybir as mybir
from concourse import bacc
from concourse.bass import ds
from concourse.bass_utils import run_bass_kernel_spmd
from concourse.tile import TileContext

F32 = mybir.dt.float32
FP16 = mybir.dt.float16
ALU = mybir.AluOpType

SIGMAS = [-1.075177135e-01, -2.867541926e-01, -7.649643581e-01,
          -2.041943548e+00, -5.459703523e+00, -1.466292403e+01,
          -3.984730093e+01, -1.117281157e+02]
C0 = -7.515097797582915
C_D = [2.244758402e-01, 4.177689340e-01, 1.223435853e+00, 3.198497940e+00,
       8.493234593e+00, 2.377242020e+01, 5.954158133e+01, 1.994069867e+02,
       4.867994757e+02]

K_STEPS = 8
N_MAT = 1024
BLK = 16
NPAIR = BLK // 2
INTERLEAVE = 4
QF = 4     # rounds >= QF use quad-tiled matmuls on the deck (no arena)


def _arena_views(AR):
    top = AR[0:64, :].rearrange("p (b c) -> p b c", c=128)[:, :, 0:64]
    bot = AR[64:128, :].rearrange("p (b c) -> p b c", c=128)[:, :, 64:128]
    return top, bot


def _mm_sq_blk(nc, psum, AR, RHS, start=True, stop=True):
    for p in range(NPAIR):
        nc.tensor.matmul(psum[0:128, ds(64 * p, 64)],
                         AR[0:128, ds(128 * p, 128)],
                         RHS[0:128, ds(64 * p, 64)],
                         start=start, stop=stop, skip_group_check=True)


def _mm_sq_quad(nc, psum, LHS, RHS, start=True, stop=True):
    """Per-pair quadrant matmuls reading deck layout directly:
    top matrix in partitions 0:64, bottom in 64:128."""
    for p in range(NPAIR):
        nc.tensor.matmul(psum[0:64, ds(64 * p, 64)],
                         LHS[0:64, ds(64 * p, 64)],
                         RHS[0:64, ds(64 * p, 64)],
                         start=start, stop=stop, tile_position=(0, 0),
                         skip_group_check=True)
        nc.tensor.matmul(psum[64:128, ds(64 * p, 64)],
                         LHS[64:128, ds(64 * p, 64)],
                         RHS[64:128, ds(64 * p, 64)],
                         start=start, stop=stop, tile_position=(64, 64),
                         skip_group_check=True)


def build_nc(n_mat=N_MAT, unroll=False, nrep=1):
    nc = bacc.Bacc("TRN2", target_bir_lowering=False, debug=False,
                   num_devices=8)
    PH_d = nc.dram_tensor("PH", [n_mat, 64, 64], FP16,
                          kind="ExternalInput").ap()
    O_d = nc.dram_tensor("OUT", [n_mat, 64, 64], F32,
                         kind="ExternalOutput").ap()
    # CIH: fp16 diag stationaries: c_0..c_8 then C0 (10 blocks of 128).
    # IDT: fp16 block-diag identity pattern (moving operand for C0*I).
    CIH_d = nc.dram_tensor("CIH", [128, 128 * (K_STEPS + 2)], FP16,
                           kind="ExternalInput").ap()
    IDT_d = nc.dram_tensor("IDT", [128, 512], FP16,
                           kind="ExternalInput").ap()
    with TileContext(nc) as tc:
        with (
            tc.tile_pool(name="consts", bufs=1) as cpool,
            tc.tile_pool(name="work", bufs=2) as pool,
            tc.tile_pool(name="psum", bufs=1, space=bass.MemorySpace.PSUM) as pspool,
            tc.tile_pool(name="psacc", bufs=1, space=bass.MemorySpace.PSUM) as paccpool,
        ):
            CIH = cpool.tile([128, 128 * (K_STEPS + 2)], FP16)
            nc.sync.dma_start(CIH[:], CIH_d)
            IDT = cpool.tile([128, 512], FP16)
            nc.sync.dma_start(IDT[:], IDT_d)

            arenas = []
            for par in range(INTERLEAVE):
                group = []
                for k in range(2):
                    AR = cpool.tile([128, 128 * NPAIR], FP16,
                                    tag=f"AR{k}_{par}", name=f"AR{k}_{par}")
                    nc.gpsimd.memset(AR[:], 0.0)
                    group.append(AR)
                arenas.append(group)

            def emit_group(par, base):
                # -- head --
                D = pool.tile([128, 512], FP16, tag=f"D0_{par}",
                              name=f"D0_{par}")
                src = PH_d[ds(base, BLK)]
                nc.sync.dma_start(
                    D[:], src.rearrange("(b h) r c -> (h r) b c", h=2))
                ar0 = arenas[par][0]
                art, arb = _arena_views(ar0)
                ev = src.rearrange("(b h) r c -> h r b c", h=2)
                nc.sync.dma_start(art, ev[0])
                nc.sync.dma_start(arb, ev[1])
                ACC = paccpool.tile([128, 512], F32, tag=f"acc{par}",
                                    name=f"acc{par}")
                nc.tensor.matmul(ACC[:, :],
                                 CIH[:, ds(128 * (K_STEPS + 1), 128)],
                                 IDT[:, :], start=True, stop=False,
                                 skip_group_check=True)
                nc.tensor.matmul(ACC[:, :], CIH[:, ds(0, 128)], D[:, :],
                                 start=False, stop=False,
                                 skip_group_check=True)
                # -- rounds --
                for j in range(K_STEPS):
                    ps = pspool.tile([128, 512], F32, tag=f"ps{par}",
                                     name=f"ps{par}")
                    if j < QF:
                        _mm_sq_blk(nc, ps, arenas[par][j % 2], D)
                    else:
                        _mm_sq_quad(nc, ps, D, D)
                    nxt = j + 1
                    Dn = pool.tile([128, 512], FP16, tag=f"D{nxt % 2}_{par}",
                                   name=f"D{nxt % 2}_{par}")
                    nc.vector.scalar_tensor_tensor(Dn, ps, float(SIGMAS[j]),
                                                   D, ALU.mult, ALU.add)
                    D = Dn
                    if nxt < QF:
                        ARn = arenas[par][nxt % 2]
                        art, arb = _arena_views(ARn)
                        nc.scalar.mul(art, Dn[0:64, :], 1.0)
                        nc.scalar.mul(arb, Dn[64:128, :], 1.0)
                    nc.tensor.matmul(ACC[:, :], CIH[:, ds(128 * nxt, 128)],
                                     D[:, :], start=False,
                                     stop=(nxt == K_STEPS),
                                     skip_group_check=True)
                OT = pool.tile([128, 512], F32, tag=f"OT_{par}",
                               name=f"OT_{par}")
                nc.scalar.copy(OT, ACC)
                dst = O_d[ds(base, BLK)]
                nc.scalar.dma_start(
                    dst.rearrange("(b h) r c -> (h r) b c", h=2), OT[:])

            def body(m0):
                for par in range(INTERLEAVE):
                    emit_group(par, m0 + par * BLK)

            step = BLK * INTERLEAVE
            if unroll:
                for m0 in range(0, n_mat, step):
                    body(m0)
            elif nrep == 1:
                with tc.For_i(0, n_mat, step) as m0:
                    body(m0)
            else:
                with tc.For_i(0, nrep) as _r:
                    with tc.For_i(0, n_mat, step) as m0:
                        body(m0)
    nc.compile()
    return nc


def _cih_const():
    ci = np.zeros((128, 128 * (K_STEPS + 2)), np.float16)
    for j in range(K_STEPS + 1):
        for p in range(128):
            ci[p, 128 * j + p] = np.float16(C_D[j])
    for p in range(128):
        ci[p, 128 * (K_STEPS + 1) + p] = np.float16(C0)
    return ci


def _idt_const():
    idt = np.zeros((128, 512), np.float16)
    for p in range(128):
        for k in range(8):
            idt[p, 64 * k + (p % 64)] = 1.0
    return idt


def host_constants():
    return {"CIH": _cih_const(), "IDT": _idt_const()}


_NC_CACHE = {}


def kernel(P: np.ndarray) -> np.ndarray:
    P = np.asarray(P)
    B, H, N, _ = P.shape
    flath = np.ascontiguousarray(
        P.reshape(-1, N, N).astype(np.float16))
    n_cores = 8
    per = flath.shape[0] // n_cores
    if "nc" not in _NC_CACHE:
        _NC_CACHE["nc"] = build_nc()
    nc = _NC_CACHE["nc"]
    consts = host_constants()
    in_maps = [
        {"PH": flath[c * per:(c + 1) * per], **consts}
        for c in range(n_cores)
    ]
    res = run_bass_kernel_spmd(nc, in_maps, core_ids=list(range(n_cores)))
    out = np.concatenate([r["OUT"] for r in res.results], axis=0)
    return out.reshape(B, H, N, N).astype(np.float32)
